# revision 5
# baseline (speedup 1.0000x reference)
"""GCN aggregation (SpMM + linear) on 8 Trainium2 NeuronCores.

out = segment_sum(feature[adj_cols] * adj_vals, adj_rows) @ W.T

Architecture (v3 "quad-window scan"):
- Destination rows sharded contiguously across 8 cores (12500 rows/core).
- The full feature table lives in SBUF as bf16 pairs: tab[128, 25001, 2],
  partition quarter g holds source window g (25000 rows), partition p holds
  emb dims (2*(p%32), 2*(p%32)+1). Column 25000 is zero (pad target).
- Edges are split per (core, window) into 4 streams, sorted by destination,
  and chunked into 66 blocks of 192 destinations. One gpsimd ap_gather per
  chunk fetches all 4 streams' source rows at once (per-Q7-core index
  lists), ~0.4ns/idx from SBUF — no DMA descriptors.
- Aggregation via DVE tensor_tensor_scan: state = d0*state + d1 where
  d1 = raw gathered features and d0 = 0 at segment starts else
  val_prev/val_cur (folds the edge-value multiply into the scan; the
  segment sum scaled by 1/val_end appears at each segment's last slot).
  A second small ap_gather picks the per-destination boundary values;
  one subtract + one multiply by val_end yields the per-dest aggregate.
- Final linear: psum[64, d] = sum_p W[o, emb(p,j)] * agg[p, j, d] over all
  128 partitions and 2 planes (absorbs the cross-window reduction), then
  out^T is DMAed out and transposed on the host.
"""

import os
import sys
import types

import numpy as np
import ml_dtypes

# ---------------------------------------------------------------- constants
N_NODES = 100000
N_EDGES = 1600000
EMB = 64
NC = 8
NPC = N_NODES // NC            # 12500 destination rows per core
NWIN = 4
WSZ = N_NODES // NWIN          # 25000 source rows per window
ZCOL = WSZ                     # zero-filled pad column in the table
NELEM = WSZ + 1
DBLK = 192                     # destinations per chunk per stream
NCH = (NPC + DBLK - 1) // DBLK # 66 chunks
AGGW = NCH * DBLK              # 12672 aggregate columns
BL = 2 * (DBLK + 1)            # 386 boundary idxs per chunk per stream
BPAD = 400                     # padded to %16
CORE_IDS = list(range(NC))

LAST_EXEC_NS = None            # filled when GCN_TRACE=1

_BF16 = ml_dtypes.bfloat16


# ------------------------------------------------------------- env plumbing
def _install_axon_ntff_shim():
    """bass_utils' axon trace path imports antenv.axon_hooks, which the
    container image lacks; wire it to the ctypes hook in trn_agent_boot."""
    if "antenv.axon_hooks" in sys.modules:
        return
    try:
        import trn_agent_boot.trn_boot as tb

        hook = tb._ntff_profile_via_ctypes("/opt/axon/libaxon_pjrt.so")
    except Exception:
        hook = None
    mod = types.ModuleType("antenv.axon_hooks")
    mod.get_axon_ntff_profile_hook = lambda: hook
    import antenv  # noqa: F401  (package must exist for submodule resolution)

    sys.modules["antenv.axon_hooks"] = mod


def _split_excess_waits(nc):
    """This walrus build allows at most ONE sync wait per instruction.
    Tile's scheduler freely attaches several; hoist the excess onto NoOp
    wait-carriers inserted just before the instruction (same engine, so
    engine program order preserves the blocking semantics)."""
    import bass_rust
    import concourse.mybir as mybir

    for f in nc.m.functions:
        for bb in f.blocks:
            new = []
            dirty = False
            for ins in bb.instructions:
                si = ins.sync_info
                # Pool engine-level sem waits take ~8-22us to wake on this
                # HW; hoist ALL of its waits onto sequencer-level NoOp
                # carriers (~19ns when satisfied). Other engines keep one
                # wait on the instruction (walrus limit), excess hoisted.
                hoist_all = (
                    si is not None
                    and len(si.on_wait) >= 1
                    and ins.engine == mybir.EngineType.Pool
                    and not isinstance(ins, mybir.InstNoOp)
                )
                if si is not None and (len(si.on_wait) > 1 or hoist_all):
                    waits = list(si.on_wait)
                    keep = [] if hoist_all else waits[-1:]
                    for k, w in enumerate(waits if hoist_all else waits[:-1]):
                        nop = mybir.InstNoOp(
                            name=f"{ins.name}-pw{k}", ins=[], outs=[]
                        )
                        nop.engine = ins.engine
                        nop.sync_info = bass_rust.SyncInfo(
                            on_wait=[w], on_update=[]
                        )
                        new.append(nop)
                    si.on_wait = keep
                    dirty = True
                new.append(ins)
            if dirty:
                bb.instructions = new


def _patch_bacc_compile():
    """Append the wait-splitter to Bacc.compile so it runs after every
    other lowering pass (walrus allows 1 sync wait per instruction)."""
    import concourse.bacc as bacc

    if getattr(bacc.Bacc, "_gcn_split_patched", False):
        return
    orig = bacc.Bacc.compile

    def _compile(self):
        orig(self)
        _split_excess_waits(self)

    bacc.Bacc.compile = _compile
    bacc.Bacc._gcn_split_patched = True


def _patch_tile_drain():
    """This walrus build rejects >1 sync wait on an InstDrain; split the
    Tile tail-drain's waits across multiple drain instructions."""
    import bass_rust
    import concourse.tile as tile
    from concourse.vector_clock import ScopedClock

    if getattr(tile.TileContext, "_gcn_drain_patched", False):
        return

    def _patched(self, tick_clock, wait_clock):
        nc = self.nc
        drain_inst = nc.sync.drain()
        wait_clock.add_sem_waits(
            drain_inst.ins, ScopedClock({None: tick_clock.global_clock})
        )
        si = drain_inst.ins.sync_info
        waits = list(si.on_wait)
        if len(waits) > 1:
            si.on_wait = waits[:1]
            for i in range(1, len(waits)):
                d2 = nc.sync.drain()
                d2.ins.sync_info = bass_rust.SyncInfo(
                    on_wait=waits[i : i + 1], on_update=[]
                )
        nc.all_engine_barrier()
        assert self.sems is not None
        popped = nc._tile_sem_poison_stack.pop()
        assert popped is self._sem_poison
        nc.clear_and_free_semaphores(list(self.sems.allocated().values()))
        nc.all_engine_barrier()

    tile.TileContext._drain_and_barrier = _patched
    tile.TileContext._gcn_drain_patched = True


# ---------------------------------------------------------- host preprocess
def _wrap16(lst):
    """[n] -> [16, n//16] wrapped layout (row p holds lst[p::16])."""
    n = len(lst)
    return np.ascontiguousarray(lst.reshape(n // 16, 16).T)


def _preprocess(rows, cols, vals):
    """Build per-core slot/boundary arrays and the shared chunk structure.

    Returns (meta, per_core):
      meta: n_k [NCH] slots per chunk, offs [NCH] slot offsets, TOT
      per_core: dicts with eidx [128, TOT//16] i16, d0g [4, TOT] f32,
                bidxg [128, NCH*BPAD//16] i16, vendg [4, NCH*DBLK] f32
    """
    keep = vals != 0.0
    rows, cols, vals = rows[keep], cols[keep], vals[keep]

    core = rows // NPC
    lr = rows - core * NPC
    win = cols // WSZ
    lc = (cols - win * WSZ).astype(np.int16)

    key = (core * NWIN + win) * NPC + lr
    order = np.argsort(key, kind="stable")
    core_s = core[order]
    win_s = win[order]
    lr_s = lr[order]
    lc_s = lc[order]
    vals_s = vals[order].astype(np.float32)
    key_s = key[order]

    ne = len(key_s)
    blk_s = lr_s // DBLK
    gid = (core_s * NWIN + win_s) * NCH + blk_s
    # continuous ratio chain within each (core, win, chunk) group: the scan
    # keeps a running sum of val*msg rescaled by 1/val_t; only the first
    # edge of each chunk resets (scan starts from the zero pad slot).
    gfirst_e = np.empty(ne, bool)
    gfirst_e[0] = True
    gfirst_e[1:] = gid[1:] != gid[:-1]
    prev = np.concatenate([[np.float32(1.0)], vals_s[:-1]])
    d0v = np.where(gfirst_e, np.float32(0.0), prev / vals_s).astype(np.float32)
    NG = NC * NWIN * NCH
    cnt_g = np.bincount(gid, minlength=NG).reshape(NC, NWIN, NCH)

    n_k = 1 + cnt_g.max(axis=(0, 1))
    n_k = ((n_k + 15) // 16) * 16
    offs = np.zeros(NCH, np.int64)
    offs[1:] = np.cumsum(n_k)[:-1]
    TOT = int(n_k.sum())

    # rank of each edge within its (core, win, chunk) group
    gstart_idx = np.flatnonzero(gfirst_e)
    gstart = gstart_idx[np.cumsum(gfirst_e) - 1]
    rank = np.arange(ne) - gstart
    slot = offs[blk_s] + 1 + rank

    sidx = np.full((NC, NWIN, TOT), ZCOL, np.int16)
    sd0 = np.zeros((NC, NWIN, TOT), np.float32)
    vslot = np.zeros((NC, NWIN, TOT), np.float32)
    sidx[core_s, win_s, slot] = lc_s
    sd0[core_s, win_s, slot] = d0v
    vslot[core_s, win_s, slot] = vals_s

    # per-destination segment ends (local to chunk)
    din_s = lr_s - blk_s * DBLK
    dkey = gid * DBLK + din_s
    cnt_d = np.bincount(dkey, minlength=NG * DBLK).reshape(NC, NWIN, NCH, DBLK)
    ends = np.cumsum(cnt_d, axis=3)  # [NC, NWIN, NCH, DBLK]

    # val at each boundary slot (last edge of dest j; for empty dests this
    # duplicates the previous boundary, making the H-difference zero)
    gsl = offs[None, None, :, None] + ends
    ci = np.arange(NC)[:, None, None, None]
    wi = np.arange(NWIN)[None, :, None, None]
    vend = vslot[ci, wi, gsl]  # [NC, NWIN, NCH, DBLK]
    # vext[i]: value multiplying G at boundary i (i=0 -> pad slot, G=0)
    vext = np.concatenate(
        [np.zeros((NC, NWIN, NCH, 1), np.float32), vend.astype(np.float32)],
        axis=3,
    )  # [.., DBLK+1]

    # boundary idx lists: scan positions of [pad, e_0..e_191] as (2e, 2e+1)
    E = np.concatenate(
        [np.zeros((NC, NWIN, NCH, 1), np.int64), ends], axis=3
    )  # [.., DBLK+1]
    bp = np.stack([2 * E, 2 * E + 1], axis=4).reshape(NC, NWIN, NCH, BL)
    bpairs = np.zeros((NC, NWIN, NCH, BPAD), np.int16)
    bpairs[:, :, :, :BL] = bp

    # packed per-chunk stage layout (bytes per partition):
    #   [0, NM*4)            d0 f32 (replicated per 32-row window group)
    #   [NM*4, NM*4+800)     vext f32 [200] (cols 0..192 used)
    #   [.., +NM//8)         edge idx i16 wrapped [NM//16]
    #   [.., +52)            boundary idx i16 wrapped [25] + 2B pad
    NM = int(max(n_k))
    OFF_VX = NM * 4
    OFF_EI = OFF_VX + 800
    OFF_BI = OFF_EI + NM // 8
    SB = OFF_BI + 52
    SB = ((SB + 3) // 4) * 4

    wrow = np.arange(128) // 32
    per_core = []
    for c in range(NC):
        stage = np.zeros((128, NCH, SB), np.uint8)
        d0rep = np.zeros((128, NM), np.float32)
        vxrep = np.zeros((128, 200), np.float32)
        for k in range(NCH):
            o = offs[k]
            n = int(n_k[k])
            d0rep[:, :] = 0.0
            d0rep[:, 0:n] = sd0[c][wrow, o : o + n]
            stage[:, k, 0 : NM * 4] = d0rep.view(np.uint8)
            vxrep[:, :] = 0.0
            vxrep[:, 0 : DBLK + 1] = vext[c][wrow, k, :]
            stage[:, k, OFF_VX : OFF_VX + 800] = vxrep.view(np.uint8)
            for w in range(NWIN):
                wr = _wrap16(sidx[c, w, o : o + n]).view(np.uint8)
                bw = _wrap16(bpairs[c, w, k]).view(np.uint8)
                for h in range(2):
                    r0 = 32 * w + 16 * h
                    stage[r0 : r0 + 16, k, OFF_EI : OFF_EI + n // 8] = wr[
                        :, : n // 8
                    ]
                    stage[r0 : r0 + 16, k, OFF_BI : OFF_BI + 50] = bw
        per_core.append({"stage": np.ascontiguousarray(
            stage.reshape(128, NCH * SB))})

    meta = {"n_k": [int(x) for x in n_k], "offs": [int(x) for x in offs],
            "TOT": TOT, "NM": NM, "SB": SB,
            "OFF_VX": OFF_VX, "OFF_EI": OFF_EI, "OFF_BI": OFF_BI}
    return meta, per_core


# ------------------------------------------------------------- device build
def _build_nc(meta):
    import concourse.bacc as bacc
    import concourse.mybir as mybir
    import concourse.tile as tile

    _patch_tile_drain()
    _patch_bacc_compile()

    n_k = meta["n_k"]
    NM = meta["NM"]
    SB = meta["SB"]
    OFF_VX = meta["OFF_VX"]
    OFF_EI = meta["OFF_EI"]
    OFF_BI = meta["OFF_BI"]
    assert NM <= 1024, NM

    f32 = mybir.dt.float32
    bf16 = mybir.dt.bfloat16
    i16 = mybir.dt.int16
    u8 = mybir.dt.uint8

    nc = bacc.Bacc(None, target_bir_lowering=False, debug=False, num_swdge_queues=4)
    tab_d = nc.declare_dram_parameter("tab", [128, NELEM, 2], bf16, isOutput=False)
    stage_d = nc.declare_dram_parameter("stage", [128, NCH * SB], u8, isOutput=False)
    wl_d = nc.declare_dram_parameter("wl", [128, 2, EMB], bf16, isOutput=False)
    out_d = nc.declare_dram_parameter("out", [EMB, AGGW], f32, isOutput=True)

    with tile.TileContext(nc) as tc:
        with (
            tc.tile_pool(name="consts", bufs=1) as cpool,
            tc.tile_pool(name="stage", bufs=5) as stpool,
            tc.tile_pool(name="msgs", bufs=3) as mpool,
            tc.tile_pool(name="sout", bufs=2) as spool,
            tc.tile_pool(name="G", bufs=2) as gpool,
            tc.tile_pool(name="H", bufs=2) as hpool,
            tc.tile_pool(name="wps", bufs=2, space="PSUM") as wpspool,
            tc.tile_pool(name="outb", bufs=2) as opool,
        ):
            tab = cpool.tile([128, NELEM, 2], bf16, tag="tab")
            nc.sync.dma_start(tab[:], tab_d[:])
            wl = cpool.tile([128, 2, EMB], bf16, tag="wl")
            nc.sync.dma_start(wl[:], wl_d[:])
            agg = cpool.tile([128, 2, AGGW], bf16, tag="agg")

            for k in range(NCH):
                n = n_k[k]
                stage = stpool.tile([128, SB], u8, tag="stage")
                nc.sync.dma_start(stage[:], stage_d[:, k * SB : (k + 1) * SB])
                d0v = stage[:, 0 : n * 4].bitcast(f32)
                vxv = stage[:, OFF_VX : OFF_VX + 800].bitcast(f32)
                eiv = stage[:, OFF_EI : OFF_EI + n // 8].bitcast(i16)
                biv = stage[:, OFF_BI : OFF_BI + 50].bitcast(i16)

                msgs = mpool.tile([128, NM, 2], bf16, tag="msgs")
                nc.gpsimd.ap_gather(
                    msgs[:, 0:n, :],
                    tab[:],
                    eiv,
                    channels=128,
                    num_elems=NELEM,
                    d=2,
                    num_idxs=n,
                )
                sout = spool.tile([128, NM, 2], f32, tag="sout")
                for j in range(2):
                    nc.vector.tensor_tensor_scan(
                        sout[:, 0:n, j],
                        d0v,
                        msgs[:, 0:n, j],
                        0.0,
                        mybir.AluOpType.mult,
                        mybir.AluOpType.add,
                    )
                G = gpool.tile([128, BPAD // 2, 2], f32, tag="G")
                nc.gpsimd.ap_gather(
                    G[:],
                    sout[:, 0:n, :],
                    biv,
                    channels=128,
                    num_elems=2 * n,
                    d=1,
                    num_idxs=BPAD,
                )
                H = hpool.tile([128, BPAD // 2, 2], f32, tag="H")
                for j in range(2):
                    nc.vector.tensor_tensor(
                        H[:, 0 : DBLK + 1, j],
                        G[:, 0 : DBLK + 1, j],
                        vxv[:, 0 : DBLK + 1],
                        mybir.AluOpType.mult,
                    )
                for j in range(2):
                    nc.vector.tensor_tensor(
                        agg[:, j, k * DBLK : (k + 1) * DBLK],
                        H[:, 1 : DBLK + 1, j],
                        H[:, 0:DBLK, j],
                        mybir.AluOpType.subtract,
                    )

            # final linear: psum[o, d] = sum_{p,j} wl[p, j, o] * agg[p, j, d]
            CH = 512
            pos = 0
            while pos < AGGW:
                ch = min(CH, AGGW - pos)
                wps = wpspool.tile([EMB, CH], f32, tag="wps")
                for j in range(2):
                    nc.tensor.matmul(
                        wps[:, 0:ch],
                        wl[:, j, :],
                        agg[:, j, pos : pos + ch],
                        start=(j == 0),
                        stop=(j == 1),
                    )
                ob = opool.tile([EMB, CH], f32, tag="ob")
                nc.scalar.activation(
                    ob[:, 0:ch], wps[:, 0:ch], mybir.ActivationFunctionType.Copy
                )
                nc.sync.dma_start(out_d[:, pos : pos + ch], ob[:, 0:ch])
                pos += ch

    nc.finalize()
    return nc


# --------------------------------------------------------------- entrypoint
def kernel(adj_rows, adj_cols, adj_vals, feature, W):
    global LAST_EXEC_NS
    _install_axon_ntff_shim()

    rows = np.asarray(adj_rows).astype(np.int64)
    cols = np.asarray(adj_cols).astype(np.int64)
    vals = np.asarray(adj_vals, dtype=np.float32)
    feat = np.asarray(feature, dtype=np.float32)
    Wm = np.asarray(W, dtype=np.float32)

    # table: tab[p, s, j] = feat[25000*(p//32) + s, 2*(p%32)+j], col 25000 = 0
    featb = feat.astype(_BF16)
    tab = np.zeros((128, NELEM, 2), dtype=_BF16)
    p = np.arange(128)
    for j in range(2):
        # [128, WSZ] view: row p -> feat[WSZ*(p//32) + s, 2*(p%32)+j]
        tab[:, :WSZ, j] = featb[
            (WSZ * (p[:, None] // 32) + np.arange(WSZ)[None, :]),
            (2 * (p[:, None] % 32) + j),
        ]
    wl = np.zeros((128, 2, EMB), dtype=_BF16)
    for j in range(2):
        wl[:, j, :] = Wm[:, (2 * (p % 32) + j)].T.astype(_BF16)

    meta, per_core = _preprocess(rows, cols, vals)
    nc = _build_nc(meta)

    in_maps = []
    for c in range(NC):
        in_maps.append(
            {"tab": tab, "stage": per_core[c]["stage"], "wl": wl}
        )

    from concourse.bass_utils import run_bass_kernel_spmd

    res = run_bass_kernel_spmd(nc, in_maps, CORE_IDS)
    out = np.empty((N_NODES, EMB), np.float32)
    for c in range(NC):
        out[c * NPC : (c + 1) * NPC, :] = res.results[c]["out"][:, :NPC].T

    if os.environ.get("GCN_TRACE") == "1":
        res2 = run_bass_kernel_spmd(nc, in_maps, CORE_IDS, trace=True)
        LAST_EXEC_NS = res2.exec_time_ns

    return out


# revision 7
# speedup vs baseline: 1.4044x; 1.4044x over previous
"""GCN aggregation (SpMM + linear) on 8 Trainium2 NeuronCores.

out = segment_sum(feature[adj_cols] * adj_vals, adj_rows) @ W.T

Strategy (all sharding internal, no collectives):
- Destination rows are sharded contiguously across the 8 cores
  (12500 rows/core). Each core owns all edges whose destination lands
  in its shard, so the segment-sum is core-local.
- Edges are bucketed by source-node window (4 windows of 25000 rows so
  gather indices fit int16) and by 128-row destination block, then
  padded to 128-edge tiles. The tile structure is made identical across
  cores (max over cores per (block, window)) because the NEFF is SPMD.
- Per tile: dma_gather pulls the 128 source rows (256B each) from a
  bf16-padded feature table in HBM straight into SBUF partitions;
  the DVE builds a val-scaled one-hot [128 edges, 128 dests] with one
  tensor_scalar (iota == ldest) * val; the PE contracts over edges:
  psum[emb, dest] += msgs^T @ valhot, accumulated per (block, window).
- Per-window aggregates (bf16) are combined inside the final W matmul
  (4 accumulating matmuls per output chunk), producing out^T in PSUM;
  out^T [64, 12500] f32 is DMAed out and transposed on the host.
"""

import os
import sys
import types

import numpy as np
import ml_dtypes

# ---------------------------------------------------------------- constants
N_NODES = 100000
N_EDGES = 1600000
EMB = 64
NC = 8
NPC = N_NODES // NC            # 12500 destination rows per core
BLK = 128                      # destination block (one-hot width)
NBLK = (NPC + BLK - 1) // BLK  # 98 blocks (last has 84 rows)
NWIN = 4
WSZ = N_NODES // NWIN          # 25000 source rows per window (< 2^15)
PAD = 128                      # feature row padded to 128 bf16 = 256B
CT_SLOTS = 1024                # max gather-chunk size: the dma_gather Q7
# ucode caps one instruction at 1024 indices (1025+ crashes the exec unit).
CORE_IDS = list(range(NC))

LAST_EXEC_NS = None            # filled when GCN_TRACE=1

_BF16 = ml_dtypes.bfloat16


# ------------------------------------------------------------- env plumbing
def _install_axon_ntff_shim():
    """bass_utils' axon trace path imports antenv.axon_hooks, which the
    container image lacks; wire it to the ctypes hook in trn_agent_boot."""
    if "antenv.axon_hooks" in sys.modules:
        return
    try:
        import trn_agent_boot.trn_boot as tb

        hook = tb._ntff_profile_via_ctypes("/opt/axon/libaxon_pjrt.so")
    except Exception:
        hook = None
    mod = types.ModuleType("antenv.axon_hooks")
    mod.get_axon_ntff_profile_hook = lambda: hook
    import antenv  # noqa: F401  (package must exist for submodule resolution)

    sys.modules["antenv.axon_hooks"] = mod


def _split_excess_waits(nc):
    """This walrus build allows at most ONE sync wait per instruction.
    Tile's scheduler freely attaches several; hoist the excess onto NoOp
    wait-carriers inserted just before the instruction (same engine, so
    engine program order preserves the blocking semantics)."""
    import bass_rust
    import concourse.mybir as mybir

    for f in nc.m.functions:
        for bb in f.blocks:
            new = []
            dirty = False
            for ins in bb.instructions:
                si = ins.sync_info
                if si is not None and len(si.on_wait) > 1:
                    waits = list(si.on_wait)
                    for k, w in enumerate(waits[:-1]):
                        nop = mybir.InstNoOp(
                            name=f"{ins.name}-pw{k}", ins=[], outs=[]
                        )
                        nop.engine = ins.engine
                        nop.sync_info = bass_rust.SyncInfo(
                            on_wait=[w], on_update=[]
                        )
                        new.append(nop)
                    si.on_wait = waits[-1:]
                    dirty = True
                new.append(ins)
            if dirty:
                bb.instructions = new


def _patch_bacc_compile():
    """Append the wait-splitter to Bacc.compile so it runs after every
    other lowering pass (walrus allows 1 sync wait per instruction)."""
    import concourse.bacc as bacc

    if getattr(bacc.Bacc, "_gcn_split_patched", False):
        return
    orig = bacc.Bacc.compile

    def _compile(self):
        orig(self)
        _split_excess_waits(self)

    bacc.Bacc.compile = _compile
    bacc.Bacc._gcn_split_patched = True


def _patch_tile_drain():
    """This walrus build rejects >1 sync wait on an InstDrain; split the
    Tile tail-drain's waits across multiple drain instructions."""
    import bass_rust
    import concourse.tile as tile
    from concourse.vector_clock import ScopedClock

    if getattr(tile.TileContext, "_gcn_drain_patched", False):
        return

    def _patched(self, tick_clock, wait_clock):
        nc = self.nc
        drain_inst = nc.sync.drain()
        wait_clock.add_sem_waits(
            drain_inst.ins, ScopedClock({None: tick_clock.global_clock})
        )
        si = drain_inst.ins.sync_info
        waits = list(si.on_wait)
        if len(waits) > 1:
            si.on_wait = waits[:1]
            for i in range(1, len(waits)):
                d2 = nc.sync.drain()
                d2.ins.sync_info = bass_rust.SyncInfo(
                    on_wait=waits[i : i + 1], on_update=[]
                )
        nc.all_engine_barrier()
        assert self.sems is not None
        popped = nc._tile_sem_poison_stack.pop()
        assert popped is self._sem_poison
        nc.clear_and_free_semaphores(list(self.sems.allocated().values()))
        nc.all_engine_barrier()

    tile.TileContext._drain_and_barrier = _patched
    tile.TileContext._gcn_drain_patched = True


# ---------------------------------------------------------- host preprocess
def _preprocess(rows, cols, vals):
    """Build the shared SPMD tile structure and per-core slot arrays.

    Returns (meta, per_core) where
      meta: dict with stream (list of (w, b, T, tile_offset)), n_tiles,
            chunks (list of (w, t0, t1)), first_w[b], block_written[b]
      per_core: list of dicts with idx16 [128, S/16], ld [128, NT] f32,
            val [128, NT] f32
    """
    core = rows // NPC
    lr = rows - core * NPC
    b = lr // BLK
    d = (lr - b * BLK).astype(np.float32)
    w = cols // WSZ
    lidx = (cols - w * WSZ).astype(np.int16)

    # group id per edge within its core: g = b * NWIN + w
    g = (b * NWIN + w).astype(np.int64)
    NG = NBLK * NWIN
    counts = np.zeros((NC, NG), np.int64)
    for c in range(NC):
        counts[c] = np.bincount(g[core == c], minlength=NG)
    tiles_per_g = -(-counts // 128)          # ceil
    T_g = tiles_per_g.max(axis=0)            # shared structure [NG]

    # stream order: (w, b) — window-major for contiguous gather windows
    stream = []          # (w, b, T, tile_offset)
    tile_off_g = np.zeros(NG, np.int64)
    t_acc = 0
    for wi in range(NWIN):
        for bi in range(NBLK):
            gi = bi * NWIN + wi
            T = int(T_g[gi])
            if T == 0:
                continue
            stream.append((wi, bi, T, t_acc))
            tile_off_g[gi] = t_acc
            t_acc += T
    n_tiles = t_acc
    S = n_tiles * 128

    # gather chunks: window-bounded slabs of <= CT_SLOTS slots
    chunks = []
    for wi in range(NWIN):
        wt = [s for s in stream if s[0] == wi]
        if not wt:
            continue
        t0 = wt[0][3]
        t1 = wt[-1][3] + wt[-1][2]
        t = t0
        while t < t1:
            te = min(t + CT_SLOTS // 128, t1)
            chunks.append((wi, t, te))
            t = te

    first_w = {}
    for wi, bi, T, _ in stream:
        if bi not in first_w:
            first_w[bi] = wi
    # windows written per block (for memset decisions)
    written = np.zeros((NBLK, NWIN), bool)
    for wi, bi, T, _ in stream:
        written[bi, wi] = True

    slot_base_g = tile_off_g * 128

    per_core = []
    for c in range(NC):
        m = core == c
        gc = g[m]
        order = np.argsort(gc, kind="stable")
        gs = gc[order]
        # rank within group
        grp_start = np.zeros(len(gs), np.int64)
        if len(gs):
            new = np.empty(len(gs), bool)
            new[0] = True
            new[1:] = gs[1:] != gs[:-1]
            idx_new = np.nonzero(new)[0]
            grp_start = idx_new[np.cumsum(new) - 1]
        rank = np.arange(len(gs)) - grp_start
        slot = slot_base_g[gs] + rank

        sidx = np.zeros(S, np.int16)
        sld = np.zeros(S, np.float32)
        sval = np.zeros(S, np.float32)
        sidx[slot] = lidx[m][order]
        sld[slot] = d[m][order]
        sval[slot] = vals[m][order]

        idx16 = np.ascontiguousarray(
            np.tile(sidx.reshape(S // 16, 16).T, (8, 1))
        )
        ld = np.ascontiguousarray(sld.reshape(n_tiles, 128).T)
        vl = np.ascontiguousarray(sval.reshape(n_tiles, 128).T)
        per_core.append({"idx": idx16, "ld": ld, "val": vl})

    meta = {
        "stream": stream,
        "n_tiles": n_tiles,
        "chunks": chunks,
        "first_w": first_w,
        "written": written,
    }
    return meta, per_core


# ------------------------------------------------------------- device build
def _build_nc(meta, max_chunks=None, do_compute=True, do_w=True):
    import concourse.bacc as bacc
    import concourse.mybir as mybir
    import concourse.tile as tile

    _patch_tile_drain()
    _patch_bacc_compile()

    stream = meta["stream"]
    n_tiles = meta["n_tiles"]
    chunks = meta["chunks"]
    if max_chunks is not None:
        chunks = chunks[:max_chunks]
    first_w = meta["first_w"]
    written = meta["written"]
    S = n_tiles * 128

    f32 = mybir.dt.float32
    bf16 = mybir.dt.bfloat16
    i16 = mybir.dt.int16

    nc = bacc.Bacc(None, target_bir_lowering=False, debug=False, num_swdge_queues=4)
    featbf = nc.declare_dram_parameter("featbf", [N_NODES, PAD], bf16, isOutput=False)
    idx_d = nc.declare_dram_parameter("idx", [128, S // 16], i16, isOutput=False)
    ld_d = nc.declare_dram_parameter("ld", [128, n_tiles], f32, isOutput=False)
    val_d = nc.declare_dram_parameter("val", [128, n_tiles], f32, isOutput=False)
    wt_d = nc.declare_dram_parameter("wt", [EMB, EMB], bf16, isOutput=False)
    iota_d = nc.declare_dram_parameter("iota", [128, BLK], bf16, isOutput=False)
    out_d = nc.declare_dram_parameter("out", [EMB, NPC], f32, isOutput=True)

    # tile index -> (w, b, k, K) lookup for the matmul loop
    tile_info = {}
    for wi, bi, T, t0 in stream:
        for k in range(T):
            tile_info[t0 + k] = (wi, bi, k, T)

    with tile.TileContext(nc) as tc:
        with (
            tc.tile_pool(name="consts", bufs=1) as cpool,
            tc.tile_pool(name="agg", bufs=1) as apool,
            tc.tile_pool(name="gather", bufs=14) as gpool,
            tc.tile_pool(name="vh", bufs=8) as vhpool,
            tc.tile_pool(name="ps", bufs=6, space="PSUM") as pspool,
            tc.tile_pool(name="wps", bufs=2, space="PSUM") as wpspool,
            tc.tile_pool(name="outb", bufs=2) as opool,
        ):
            iota_t = cpool.tile([128, BLK], bf16, tag="iota")
            nc.sync.dma_start(iota_t[:], iota_d[:])
            wt_t = cpool.tile([EMB, EMB], bf16, tag="wt")
            nc.sync.dma_start(wt_t[:], wt_d[:])
            ix_all = cpool.tile([128, S // 16], i16, tag="ixall")
            nc.sync.dma_start(ix_all[:], idx_d[:])
            ld_all = cpool.tile([128, n_tiles], f32, tag="ldall")
            nc.sync.dma_start(ld_all[:], ld_d[:])
            vl_all = cpool.tile([128, n_tiles], f32, tag="vlall")
            nc.sync.dma_start(vl_all[:], val_d[:])

            aggw = []
            for wi in range(NWIN):
                a = apool.tile([EMB, NBLK * BLK], bf16, tag=f"aggw{wi}")
                aggw.append(a)
                # zero slices never written by the stream
                holes = [bi for bi in range(NBLK) if not written[bi, wi]]
                if max_chunks is not None or not do_compute:
                    holes = list(range(NBLK))
                if len(holes) == NBLK:
                    nc.vector.memset(a[:], 0.0)
                else:
                    for bi in holes:
                        nc.vector.memset(a[:, bi * BLK : (bi + 1) * BLK], 0.0)

            psum_cur = None
            for ci, (wi, t0, t1) in enumerate(chunks):
                ctiles = t1 - t0
                cs = ctiles * 128
                g = gpool.tile([128, ctiles, PAD], bf16, tag="g")
                nc.gpsimd.dma_gather(
                    g[:, :, :],
                    featbf[wi * WSZ : (wi + 1) * WSZ, :],
                    ix_all[:, t0 * 8 : t0 * 8 + cs // 16],
                    num_idxs=cs,
                    num_idxs_reg=cs,
                    elem_size=PAD,
                    queue_num=ci % 4,
                )

                for t in range(t0, t1):
                    if not do_compute:
                        break
                    twi, bi, k, K = tile_info[t]
                    vh = vhpool.tile([128, BLK], bf16, tag="vh")
                    nc.vector.tensor_scalar(
                        vh[:],
                        iota_t[:],
                        ld_all[:, t : t + 1],
                        vl_all[:, t : t + 1],
                        mybir.AluOpType.is_equal,
                        mybir.AluOpType.mult,
                    )
                    if k == 0:
                        psum_cur = pspool.tile([EMB, BLK], f32, tag="ps")
                    nc.tensor.matmul(
                        psum_cur[:],
                        g[:, t - t0, 0:EMB],
                        vh[:],
                        start=(k == 0),
                        stop=(k == K - 1),
                    )
                    if k == K - 1:
                        nc.scalar.activation(
                            aggw[twi][:, bi * BLK : (bi + 1) * BLK],
                            psum_cur[:],
                            mybir.ActivationFunctionType.Copy,
                        )

            # final W transform: out^T[o, dest] = sum_w W.T^T @ aggw[w]
            CH = 512
            pos = 0
            while do_w and pos < NPC:
                ch = min(CH, NPC - pos)
                wps = wpspool.tile([EMB, CH], f32, tag="wps")
                for wi in range(NWIN):
                    nc.tensor.matmul(
                        wps[:, 0:ch],
                        wt_t[:],
                        aggw[wi][:, pos : pos + ch],
                        start=(wi == 0),
                        stop=(wi == NWIN - 1),
                    )
                ob = opool.tile([EMB, CH], f32, tag="ob")
                nc.scalar.activation(
                    ob[:, 0:ch], wps[:, 0:ch], mybir.ActivationFunctionType.Copy
                )
                nc.sync.dma_start(out_d[:, pos : pos + ch], ob[:, 0:ch])
                pos += ch

    nc.finalize()
    return nc


# --------------------------------------------------------------- entrypoint
def kernel(adj_rows, adj_cols, adj_vals, feature, W):
    global LAST_EXEC_NS
    _install_axon_ntff_shim()

    rows = np.asarray(adj_rows).astype(np.int64)
    cols = np.asarray(adj_cols).astype(np.int64)
    vals = np.asarray(adj_vals, dtype=np.float32)
    feat = np.asarray(feature, dtype=np.float32)
    Wm = np.asarray(W, dtype=np.float32)

    featbf = np.zeros((N_NODES, PAD), dtype=_BF16)
    featbf[:, :EMB] = feat.astype(_BF16)
    wt = np.ascontiguousarray(Wm.T).astype(_BF16)
    iota = np.broadcast_to(
        np.arange(BLK, dtype=np.float32), (128, BLK)
    ).astype(_BF16)
    iota = np.ascontiguousarray(iota)

    meta, per_core = _preprocess(rows, cols, vals)
    nc = _build_nc(meta)

    in_maps = []
    for c in range(NC):
        in_maps.append(
            {
                "featbf": featbf,
                "idx": per_core[c]["idx"],
                "ld": per_core[c]["ld"],
                "val": per_core[c]["val"],
                "wt": wt,
                "iota": iota,
            }
        )

    from concourse.bass_utils import run_bass_kernel_spmd

    res = run_bass_kernel_spmd(nc, in_maps, CORE_IDS)
    out = np.empty((N_NODES, EMB), np.float32)
    for c in range(NC):
        out[c * NPC : (c + 1) * NPC, :] = res.results[c]["out"].T

    if os.environ.get("GCN_TRACE") == "1":
        res2 = run_bass_kernel_spmd(nc, in_maps, CORE_IDS, trace=True)
        LAST_EXEC_NS = res2.exec_time_ns

    return out



# revision 8
# speedup vs baseline: 1.6053x; 1.1431x over previous
"""GCN aggregation (SpMM + linear) on 8 Trainium2 NeuronCores.

out = segment_sum(feature[adj_cols] * adj_vals, adj_rows) @ W.T

Strategy (all sharding internal, no collectives):
- Destination rows are sharded contiguously across the 8 cores
  (12500 rows/core). Each core owns all edges whose destination lands
  in its shard, so the segment-sum is core-local.
- Edges are bucketed by source-node window (4 windows of 25000 rows so
  gather indices fit int16) and by 128-row destination block, then
  padded to 128-edge tiles. The tile structure is made identical across
  cores (max over cores per (block, window)) because the NEFF is SPMD.
- Per tile: dma_gather pulls the 128 source rows (256B each) from a
  bf16-padded feature table in HBM straight into SBUF partitions;
  the DVE builds a val-scaled one-hot [128 edges, 128 dests] with one
  tensor_scalar (iota == ldest) * val; the PE contracts over edges:
  psum[emb, dest] += msgs^T @ valhot, accumulated per (block, window).
- Per-window aggregates (bf16) are combined inside the final W matmul
  (4 accumulating matmuls per output chunk), producing out^T in PSUM;
  out^T [64, 12500] f32 is DMAed out and transposed on the host.
"""

import os
import sys
import types

import numpy as np
import ml_dtypes

# ---------------------------------------------------------------- constants
N_NODES = 100000
N_EDGES = 1600000
EMB = 64
NC = 8
NPC = N_NODES // NC            # 12500 destination rows per core
BLK = 128                      # destination block (one-hot width)
NBLK = (NPC + BLK - 1) // BLK  # 98 blocks (last has 84 rows)
NWIN = 4
WSZ = N_NODES // NWIN          # 25000 source rows per window (< 2^15)
PAD = 128                      # feature row padded to 128 bf16 = 256B
CT_SLOTS = 1024                # max gather-chunk size: the dma_gather Q7
# ucode caps one instruction at 1024 indices (1025+ crashes the exec unit).
CORE_IDS = list(range(NC))

LAST_EXEC_NS = None            # filled when GCN_TRACE=1

_BF16 = ml_dtypes.bfloat16


# ------------------------------------------------------------- env plumbing
def _install_axon_ntff_shim():
    """bass_utils' axon trace path imports antenv.axon_hooks, which the
    container image lacks; wire it to the ctypes hook in trn_agent_boot."""
    if "antenv.axon_hooks" in sys.modules:
        return
    try:
        import trn_agent_boot.trn_boot as tb

        hook = tb._ntff_profile_via_ctypes("/opt/axon/libaxon_pjrt.so")
    except Exception:
        hook = None
    mod = types.ModuleType("antenv.axon_hooks")
    mod.get_axon_ntff_profile_hook = lambda: hook
    import antenv  # noqa: F401  (package must exist for submodule resolution)

    sys.modules["antenv.axon_hooks"] = mod


def _split_excess_waits(nc):
    """This walrus build allows at most ONE sync wait per instruction.
    Tile's scheduler freely attaches several; hoist the excess onto NoOp
    wait-carriers inserted just before the instruction (same engine, so
    engine program order preserves the blocking semantics)."""
    import bass_rust
    import concourse.mybir as mybir

    for f in nc.m.functions:
        for bb in f.blocks:
            new = []
            dirty = False
            for ins in bb.instructions:
                si = ins.sync_info
                if si is not None and len(si.on_wait) > 1:
                    waits = list(si.on_wait)
                    for k, w in enumerate(waits[:-1]):
                        nop = mybir.InstNoOp(
                            name=f"{ins.name}-pw{k}", ins=[], outs=[]
                        )
                        nop.engine = ins.engine
                        nop.sync_info = bass_rust.SyncInfo(
                            on_wait=[w], on_update=[]
                        )
                        new.append(nop)
                    si.on_wait = waits[-1:]
                    dirty = True
                new.append(ins)
            if dirty:
                bb.instructions = new


def _patch_bacc_compile():
    """Append the wait-splitter to Bacc.compile so it runs after every
    other lowering pass (walrus allows 1 sync wait per instruction)."""
    import concourse.bacc as bacc

    if getattr(bacc.Bacc, "_gcn_split_patched", False):
        return
    orig = bacc.Bacc.compile

    def _compile(self):
        orig(self)
        _split_excess_waits(self)

    bacc.Bacc.compile = _compile
    bacc.Bacc._gcn_split_patched = True


def _patch_tile_drain():
    """This walrus build rejects >1 sync wait on an InstDrain; split the
    Tile tail-drain's waits across multiple drain instructions."""
    import bass_rust
    import concourse.tile as tile
    from concourse.vector_clock import ScopedClock

    if getattr(tile.TileContext, "_gcn_drain_patched", False):
        return

    def _patched(self, tick_clock, wait_clock):
        nc = self.nc
        drain_inst = nc.sync.drain()
        wait_clock.add_sem_waits(
            drain_inst.ins, ScopedClock({None: tick_clock.global_clock})
        )
        si = drain_inst.ins.sync_info
        waits = list(si.on_wait)
        if len(waits) > 1:
            si.on_wait = waits[:1]
            for i in range(1, len(waits)):
                d2 = nc.sync.drain()
                d2.ins.sync_info = bass_rust.SyncInfo(
                    on_wait=waits[i : i + 1], on_update=[]
                )
        nc.all_engine_barrier()
        assert self.sems is not None
        popped = nc._tile_sem_poison_stack.pop()
        assert popped is self._sem_poison
        nc.clear_and_free_semaphores(list(self.sems.allocated().values()))
        nc.all_engine_barrier()

    tile.TileContext._drain_and_barrier = _patched
    tile.TileContext._gcn_drain_patched = True


# ---------------------------------------------------------- host preprocess
def _preprocess(rows, cols, vals):
    """Build the shared SPMD tile structure and per-core slot arrays.

    Returns (meta, per_core) where
      meta: dict with stream (list of (w, b, T, tile_offset)), n_tiles,
            chunks (list of (w, t0, t1)), first_w[b], block_written[b]
      per_core: list of dicts with idx16 [128, S/16], ld [128, NT] f32,
            val [128, NT] f32
    """
    core = rows // NPC
    lr = rows - core * NPC
    b = lr // BLK
    d = (lr - b * BLK).astype(np.float32)
    w = cols // WSZ
    lidx = (cols - w * WSZ).astype(np.int16)

    # group id per edge within its core: g = b * NWIN + w
    g = (b * NWIN + w).astype(np.int64)
    NG = NBLK * NWIN
    counts = np.zeros((NC, NG), np.int64)
    for c in range(NC):
        counts[c] = np.bincount(g[core == c], minlength=NG)
    tiles_per_g = -(-counts // 128)          # ceil
    T_g = tiles_per_g.max(axis=0)            # shared structure [NG]

    # stream order: (w, b) — window-major for contiguous gather windows
    stream = []          # (w, b, T, tile_offset)
    tile_off_g = np.zeros(NG, np.int64)
    t_acc = 0
    for wi in range(NWIN):
        for bi in range(NBLK):
            gi = bi * NWIN + wi
            T = int(T_g[gi])
            if T == 0:
                continue
            stream.append((wi, bi, T, t_acc))
            tile_off_g[gi] = t_acc
            t_acc += T
    n_tiles = t_acc
    S = n_tiles * 128

    # gather chunks: window-bounded slabs of <= CT_SLOTS slots
    chunks = []
    for wi in range(NWIN):
        wt = [s for s in stream if s[0] == wi]
        if not wt:
            continue
        t0 = wt[0][3]
        t1 = wt[-1][3] + wt[-1][2]
        t = t0
        while t < t1:
            te = min(t + CT_SLOTS // 128, t1)
            chunks.append((wi, t, te))
            t = te

    first_w = {}
    for wi, bi, T, _ in stream:
        if bi not in first_w:
            first_w[bi] = wi
    # windows written per block (for memset decisions)
    written = np.zeros((NBLK, NWIN), bool)
    for wi, bi, T, _ in stream:
        written[bi, wi] = True

    slot_base_g = tile_off_g * 128

    per_core = []
    for c in range(NC):
        m = core == c
        gc = g[m]
        order = np.argsort(gc, kind="stable")
        gs = gc[order]
        # rank within group
        grp_start = np.zeros(len(gs), np.int64)
        if len(gs):
            new = np.empty(len(gs), bool)
            new[0] = True
            new[1:] = gs[1:] != gs[:-1]
            idx_new = np.nonzero(new)[0]
            grp_start = idx_new[np.cumsum(new) - 1]
        rank = np.arange(len(gs)) - grp_start
        slot = slot_base_g[gs] + rank

        sidx = np.zeros(S, np.int16)
        sld = np.zeros(S, np.float32)
        sval = np.zeros(S, np.float32)
        sidx[slot] = lidx[m][order]
        sld[slot] = d[m][order]
        sval[slot] = vals[m][order]

        idx16 = np.ascontiguousarray(
            np.tile(sidx.reshape(S // 16, 16).T, (8, 1))
        )
        ld = np.ascontiguousarray(sld.reshape(n_tiles, 128).T)
        vl = np.ascontiguousarray(sval.reshape(n_tiles, 128).T)
        per_core.append({"idx": idx16, "ld": ld, "val": vl})

    meta = {
        "stream": stream,
        "n_tiles": n_tiles,
        "chunks": chunks,
        "first_w": first_w,
        "written": written,
    }
    return meta, per_core


# ------------------------------------------------------------- device build
def _build_nc(meta, max_chunks=None, do_compute=True, do_w=True):
    import concourse.bacc as bacc
    import concourse.mybir as mybir
    import concourse.tile as tile

    _patch_tile_drain()
    _patch_bacc_compile()

    stream = meta["stream"]
    n_tiles = meta["n_tiles"]
    chunks = meta["chunks"]
    if max_chunks is not None:
        chunks = chunks[:max_chunks]
    first_w = meta["first_w"]
    written = meta["written"]
    S = n_tiles * 128

    f32 = mybir.dt.float32
    bf16 = mybir.dt.bfloat16
    i16 = mybir.dt.int16

    nc = bacc.Bacc(None, target_bir_lowering=False, debug=False, num_swdge_queues=4)
    featbf = nc.declare_dram_parameter("featbf", [N_NODES, PAD], bf16, isOutput=False)
    idx_d = nc.declare_dram_parameter("idx", [128, S // 16], i16, isOutput=False)
    ld_d = nc.declare_dram_parameter("ld", [128, n_tiles], f32, isOutput=False)
    val_d = nc.declare_dram_parameter("val", [128, n_tiles], f32, isOutput=False)
    wt_d = nc.declare_dram_parameter("wt", [EMB, EMB], bf16, isOutput=False)
    iota_d = nc.declare_dram_parameter("iota", [128, BLK], bf16, isOutput=False)
    out_d = nc.declare_dram_parameter("out", [EMB, NPC], f32, isOutput=True)

    # tile index -> (w, b, k, K) lookup for the matmul loop
    tile_info = {}
    for wi, bi, T, t0 in stream:
        for k in range(T):
            tile_info[t0 + k] = (wi, bi, k, T)

    with tile.TileContext(nc) as tc:
        with (
            tc.tile_pool(name="consts", bufs=1) as cpool,
            tc.tile_pool(name="agg", bufs=1) as apool,
            tc.tile_pool(name="gather", bufs=8) as gpool,
            tc.tile_pool(name="vh", bufs=4) as vhpool,
            tc.tile_pool(name="ps", bufs=6, space="PSUM") as pspool,
            tc.tile_pool(name="wps", bufs=2, space="PSUM") as wpspool,
            tc.tile_pool(name="outb", bufs=2) as opool,
        ):
            iota_t = cpool.tile([128, BLK], bf16, tag="iota")
            nc.sync.dma_start(iota_t[:], iota_d[:])
            wt_t = cpool.tile([EMB, EMB], bf16, tag="wt")
            nc.sync.dma_start(wt_t[:], wt_d[:])
            ix_all = cpool.tile([128, S // 16], i16, tag="ixall")
            nc.sync.dma_start(ix_all[:], idx_d[:])
            ld_all = cpool.tile([128, n_tiles], f32, tag="ldall")
            nc.sync.dma_start(ld_all[:], ld_d[:])
            vl_all = cpool.tile([128, n_tiles], f32, tag="vlall")
            nc.sync.dma_start(vl_all[:], val_d[:])

            aggw = []
            for wi in range(NWIN):
                a = apool.tile([EMB, NBLK * BLK], bf16, tag=f"aggw{wi}")
                aggw.append(a)
                # zero slices never written by the stream
                holes = [bi for bi in range(NBLK) if not written[bi, wi]]
                if max_chunks is not None or not do_compute:
                    holes = list(range(NBLK))
                if len(holes) == NBLK:
                    nc.vector.memset(a[:], 0.0)
                else:
                    for bi in holes:
                        nc.vector.memset(a[:, bi * BLK : (bi + 1) * BLK], 0.0)

            psum_cur = None
            for ci, (wi, t0, t1) in enumerate(chunks):
                ctiles = t1 - t0
                cs = ctiles * 128
                g = gpool.tile([128, ctiles, PAD], bf16, tag="g")
                nc.gpsimd.dma_gather(
                    g[:, :, :],
                    featbf[wi * WSZ : (wi + 1) * WSZ, :],
                    ix_all[:, t0 * 8 : t0 * 8 + cs // 16],
                    num_idxs=cs,
                    num_idxs_reg=cs,
                    elem_size=PAD,
                    queue_num=ci % 4,
                )

                for t in range(t0, t1):
                    if not do_compute:
                        break
                    twi, bi, k, K = tile_info[t]
                    vh = vhpool.tile([128, BLK], bf16, tag="vh")
                    nc.vector.tensor_scalar(
                        vh[:],
                        iota_t[:],
                        ld_all[:, t : t + 1],
                        vl_all[:, t : t + 1],
                        mybir.AluOpType.is_equal,
                        mybir.AluOpType.mult,
                    )
                    if k == 0:
                        psum_cur = pspool.tile([EMB, BLK], f32, tag="ps")
                    nc.tensor.matmul(
                        psum_cur[:],
                        g[:, t - t0, 0:EMB],
                        vh[:],
                        start=(k == 0),
                        stop=(k == K - 1),
                    )
                    if k == K - 1:
                        nc.scalar.activation(
                            aggw[twi][:, bi * BLK : (bi + 1) * BLK],
                            psum_cur[:],
                            mybir.ActivationFunctionType.Copy,
                        )

            # final W transform: out^T[o, dest] = sum_w W.T^T @ aggw[w]
            CH = 512
            pos = 0
            while do_w and pos < NPC:
                ch = min(CH, NPC - pos)
                wps = wpspool.tile([EMB, CH], f32, tag="wps")
                for wi in range(NWIN):
                    nc.tensor.matmul(
                        wps[:, 0:ch],
                        wt_t[:],
                        aggw[wi][:, pos : pos + ch],
                        start=(wi == 0),
                        stop=(wi == NWIN - 1),
                    )
                ob = opool.tile([EMB, CH], f32, tag="ob")
                nc.scalar.activation(
                    ob[:, 0:ch], wps[:, 0:ch], mybir.ActivationFunctionType.Copy
                )
                nc.sync.dma_start(out_d[:, pos : pos + ch], ob[:, 0:ch])
                pos += ch

    nc.finalize()
    return nc


# --------------------------------------------------------------- entrypoint
def kernel(adj_rows, adj_cols, adj_vals, feature, W):
    global LAST_EXEC_NS
    _install_axon_ntff_shim()

    rows = np.asarray(adj_rows).astype(np.int64)
    cols = np.asarray(adj_cols).astype(np.int64)
    vals = np.asarray(adj_vals, dtype=np.float32)
    feat = np.asarray(feature, dtype=np.float32)
    Wm = np.asarray(W, dtype=np.float32)

    featbf = np.zeros((N_NODES, PAD), dtype=_BF16)
    featbf[:, :EMB] = feat.astype(_BF16)
    wt = np.ascontiguousarray(Wm.T).astype(_BF16)
    iota = np.broadcast_to(
        np.arange(BLK, dtype=np.float32), (128, BLK)
    ).astype(_BF16)
    iota = np.ascontiguousarray(iota)

    meta, per_core = _preprocess(rows, cols, vals)
    nc = _build_nc(meta)

    in_maps = []
    for c in range(NC):
        in_maps.append(
            {
                "featbf": featbf,
                "idx": per_core[c]["idx"],
                "ld": per_core[c]["ld"],
                "val": per_core[c]["val"],
                "wt": wt,
                "iota": iota,
            }
        )

    from concourse.bass_utils import run_bass_kernel_spmd

    res = run_bass_kernel_spmd(nc, in_maps, CORE_IDS)
    out = np.empty((N_NODES, EMB), np.float32)
    for c in range(NC):
        out[c * NPC : (c + 1) * NPC, :] = res.results[c]["out"].T

    if os.environ.get("GCN_TRACE") == "1":
        res2 = run_bass_kernel_spmd(nc, in_maps, CORE_IDS, trace=True)
        LAST_EXEC_NS = res2.exec_time_ns

    return out

